# revision 35
# baseline (speedup 1.0000x reference)
"""AIMNet-style embedding kernel for 8 Trainium2 NeuronCores.

Data-parallel over the molecule batch B=8 (one molecule per core).
Host-side prep does layout transforms only (transpose ga/gr so the
contraction axis is on partitions, bf16 casts, small broadcast tables);
all FLOPs run on device.

Per-core device pipeline (molecule b):
  1. pair build:  X^T[128, 8128]  (one tensor_scalar per anchor atom i,
     split across DVE and GpSimd)
  2. combine MLP: C1^T = cw1^T @ X^T -> gelu -> G1^T ;  FP = G1^T chunks @ cw2
  3. grv:  afv^T @ grT slices  (per radial shift r)     -> Z^T k-tiles 0..15
  4. gav:  FP_k^T @ gaT k-tiles (64 accumulating steps into psum[32, 2048])
  5. embed MLP: accumulate psum[n, 512] over Z^T k-tiles (Z as stationary,
     ew1 as moving; eb1 folded in as a rank-1 matmul), gelu, PE-transpose
     A1 -> A1^T, then ew2^T @ A1^T -> AEF^T (+eb2) -> out

Stages 1/2/4 are emitted interleaved so the PE's in-order stream starts
consuming gaT tiles as soon as the first FP tiles exist, and the zt-part
of the embed accumulation runs mid-stream; only the 4 gav k-tiles of the
embed remain after the last gaT tile.
"""

import numpy as np
import ml_dtypes

import concourse.bass as bass
import concourse.mybir as mybir
import concourse.tile as tile
from concourse import bacc
from concourse.bass_utils import run_bass_kernel_spmd
from concourse.masks import make_identity

BF16NP = ml_dtypes.bfloat16
F32 = mybir.dt.float32
BF = mybir.dt.bfloat16

B, N, A = 8, 128, 64
Rr, Ra = 32, 16
P = N * (N - 1) // 2          # 8128
D = 32                        # d_pair
H, E = 512, 256
M2 = N * Ra                   # 2048 = gav output dim (r'-major: m = r'*128 + n)
G2 = N * Rr                   # 4096 = grT cols (r-major: r*128 + n)
NKT = (P + N - 1) // N        # 64 pair k-tiles (63 full + one of 64)

GELU = mybir.ActivationFunctionType.Gelu_apprx_tanh
IDENT = mybir.ActivationFunctionType.Identity
MULT = mybir.AluOpType.mult
ADD = mybir.AluOpType.add

_CACHE: dict = {}

# tunables (A/B testing); kernel defaults are the shipped config
KCFG = {
    "ga_bufs": 14,
    "pieces4": True,     # 4 strided piece-pack ops vs 16 small ones
    "gelu_split": True,  # per-chunk a1 gelu
    "out_split": True,   # per-chunk output DMA
    "w_gpsimd": True,    # bulk weights via SWDGE mid-stream (rings stay clean)
    "split32": True,     # 3:2 DVE:GpSimd pair-block split (else 2:1)
    "last_split": True,  # final DMA pair split across both rings
    "fine": False,       # 256-pair C1 chunks (faster pipeline head)
    "w_at": 3,           # loop position to issue SWDGE weight loads
    "late_init": True,   # emit ones1/ident init away from the queue head
    "lead": 2,           # chunks of slack between c1 and its gav group
    "const_scalar": False,  # consts on the scalar ring (sync ring starts with ga)
    "quad": False,       # 2MB quad gaT DMAs (4 k-tiles) instead of 1MB pairs
    "split772": False,   # 7:7:2 DVE:GpSimd:ACT pair-block split
    "pieces22": False,   # piece-pack ops split 2 DVE + 2 ACT
    "fp_batch": False,   # one psum tile + one DVE add per 4 FP chunks
    "ring22": False,     # 2-2 ring interleave for gaT pairs (else 1-1)
    "zte_at": 14,        # loop position of the zt-part embed accumulation
}


def _build_nc():
    nc = bacc.Bacc("TRN2", target_bir_lowering=False)

    gaT = nc.dram_tensor("gaT", [P, M2], BF, kind="ExternalInput")
    grT = nc.dram_tensor("grT", [N, G2], BF, kind="ExternalInput")
    # packF cols: cb1 0:1 | cb2b 1:33 | afv2 33:161 | s1 161:289 | s2 289:417 | eb2t 417:419
    packF = nc.dram_tensor("packF", [128, 419], F32, kind="ExternalInput")
    # packB cols: cw1 0:128 | cw2 128:160 | afv 160:224
    packB = nc.dram_tensor("packB", [128, 224], BF, kind="ExternalInput")
    eb1r = nc.dram_tensor("eb1r", [1, H], BF, kind="ExternalInput")
    ew1t = nc.dram_tensor("ew1t", [128, 20 * H], BF, kind="ExternalInput")
    ew2t = nc.dram_tensor("ew2t", [128, 4 * E], BF, kind="ExternalInput")
    out = nc.dram_tensor("out", [E, N], F32, kind="ExternalOutput")

    with tile.TileContext(nc) as tc:
        with (
            tc.tile_pool(name="const", bufs=1) as cp,
            tc.tile_pool(name="big", bufs=1) as bp,
            tc.tile_pool(name="ga", bufs=KCFG["ga_bufs"]) as gap,
        ):
            # ---- constants (host-packed: 3 small DMAs) ----
            cdma = nc.scalar if KCFG["const_scalar"] else nc.sync
            packFs = cp.tile([128, 419], F32)
            cdma.dma_start(out=packFs, in_=packF[:])
            packBs = cp.tile([128, 224], BF)
            cdma.dma_start(out=packBs, in_=packB[:])
            eb1rs = cp.tile([1, H], BF)
            cdma.dma_start(out=eb1rs, in_=eb1r[:])
            cb1s = packFs[:, 0:1]
            cb2bs = packFs[:, 1:33]
            afv2s = packFs[:, 33:161]
            s1s = packFs[:, 161:289]
            s2s = packFs[:, 289:417]
            eb2s = packFs[:, 417:419]
            cw1s = packBs[:, 0:128]
            cw2s = packBs[:, 128:160]
            afvs = packBs[:, 160:224]
            # bulk weights (host-pretiled: contiguous per-partition runs)
            grts = cp.tile([128, G2], BF)
            ew2s = cp.tile([128, 4, E], BF)
            ew1s = cp.tile([128, 20, H], BF)
            wdma = nc.gpsimd if KCFG["w_gpsimd"] else nc.scalar
            if not KCFG["w_gpsimd"]:
                wdma.dma_start(out=grts, in_=grT[:])
                wdma.dma_start(
                    out=ew2s, in_=ew2t[:].rearrange("p (t e) -> p t e", t=4)
                )
                wdma.dma_start(
                    out=ew1s, in_=ew1t[:].rearrange("p (t h) -> p t h", t=20)
                )

            ones1 = cp.tile([1, N], BF)
            ident = cp.tile([128, 128], BF)
            if not KCFG["late_init"]:
                nc.vector.memset(ones1, 1.0)
                make_identity(nc, ident)

            # ---- persistent intermediates ----
            xt = bp.tile([128, P], BF)           # X^T  (pair features)
            g1t = bp.tile([128, P], BF)          # gelu(C1)^T
            fps = bp.tile([128, NKT * D], BF)    # FP, k-tile q at cols [q*32, q*32+32)
            zt = bp.tile([128, 16 * N], BF)      # Z^T grv part, k-tile kt at cols kt*128
            ztg = bp.tile([128, 4, N], BF)       # Z^T gav part, packed 4 pieces/k-tile
            a1 = bp.tile([128, H], BF)           # A1 [n, h]
            a1t = bp.tile([128, 4, N], BF)       # A1^T, h-chunk ht at [:, ht, :]
            aeft = bp.tile([128, 2, N], F32)     # AEF^T chunks

            # ---- stages 1+2+4 interleaved ----
            # Pair block i (i=0..126) covers pairs (i, j) j=i+1..127 (width
            # 127-i).  Emission is pipelined per 512-pair C1 chunk, and the
            # gav matmuls for group g (FP chunks 4g..4g+3 = DMA pairs 2g,
            # 2g+1) are emitted right after the FP tiles they need, so the
            # PE's in-order stream never parks gav work behind the whole
            # pair-build chain.
            offs = np.concatenate([[0], np.cumsum(N - 1 - np.arange(N - 1))])
            next_blk = 0

            def emit_pair_blocks_until(cov):
                nonlocal next_blk
                while next_blk < N - 1 and offs[next_blk] < cov:
                    i = next_blk
                    if KCFG["split772"] and i % 16 >= 14:
                        nc.scalar.activation(
                            xt[:, offs[i]:offs[i + 1]],
                            afv2s[:, i + 1:N],
                            IDENT,
                            bias=s2s[:, i:i + 1],
                            scale=s1s[:, i:i + 1],
                        )
                        next_blk += 1
                        continue
                    if KCFG["split772"]:
                        eng = nc.gpsimd if i % 2 == 1 else nc.vector
                    elif KCFG["split32"]:
                        eng = nc.gpsimd if i % 5 in (2, 4) else nc.vector
                    else:
                        eng = nc.gpsimd if i % 3 == 2 else nc.vector
                    eng.tensor_scalar(
                        out=xt[:, offs[i]:offs[i + 1]],
                        in0=afv2s[:, i + 1:N],
                        scalar1=s1s[:, i:i + 1],
                        scalar2=s2s[:, i:i + 1],
                        op0=MULT,
                        op1=ADD,
                    )
                    next_blk += 1

            # PSUM pool lifetimes (explicit, LIFO):
            #   psGav (4 banks) spans the whole stream;
            #   psA (c1/fp shared tag, 2 banks) spans stages 1+2;
            #   psGrv (2 banks) closes after grv, freeing room for ps1 (1).
            psGav_cm = tc.tile_pool(name="psGav", bufs=1, space="PSUM")
            psGav = psGav_cm.__enter__()
            psA_cm = tc.tile_pool(name="psA", bufs=2, space="PSUM")
            psA = psA_cm.__enter__()
            psGrv_cm = tc.tile_pool(name="psGrv", bufs=1, space="PSUM")
            psGrv = psGrv_cm.__enter__()

            psg = psGav.tile([32, M2], F32)

            c1w = 256 if KCFG["fine"] else 512

            def emit_c1(pc):
                w = min(c1w, P - pc * c1w)
                ps = psA.tile([128, 512], F32, tag="c1")
                nc.tensor.matmul(
                    ps[:, 0:w], cw1s[:, :], xt[:, pc * c1w:pc * c1w + w],
                    start=True, stop=True,
                )
                nc.scalar.activation(
                    g1t[:, pc * c1w:pc * c1w + w], ps[:, 0:w], GELU,
                    bias=cb1s[:, 0:1], scale=1.0,
                )

            def emit_fp(q):
                kw = min(128, P - q * 128)
                ps = psA.tile([128, 512], F32, tag="c1")
                nc.tensor.matmul(
                    ps[0:kw, 0:D], g1t[:, q * 128:q * 128 + kw], cw2s[:, :],
                    start=True, stop=True,
                )
                nc.vector.tensor_tensor(
                    out=fps[0:kw, q * D:(q + 1) * D],
                    in0=ps[0:kw, 0:D],
                    in1=cb2bs[0:kw, :],
                    op=ADD,
                )

            def cb2_bcast(reps, rows=128):
                ap = cb2bs[0:rows, :]
                import copy as _copy
                b = bass.AP(tensor=ap.tensor, offset=ap.offset,
                            ap=[ap.ap[0], [0, reps], ap.ap[1]])
                return b

            def emit_fp_tile(g):
                # 4 FP chunks (4g..4g+3) into one psum tile, one batched add
                ps = psA.tile([128, 512], F32, tag="c1")
                for sub in range(4):
                    q = 4 * g + sub
                    kw = min(128, P - q * 128)
                    nc.tensor.matmul(
                        ps[0:kw, sub * D:(sub + 1) * D],
                        g1t[:, q * 128:q * 128 + kw], cw2s[:, :],
                        start=True, stop=True,
                    )
                if g < 15:
                    nc.vector.tensor_tensor(
                        out=fps[:, g * 128:(g + 1) * 128],
                        in0=ps[:, 0:128],
                        in1=cb2_bcast(4),
                        op=ADD,
                    )
                else:
                    nc.vector.tensor_tensor(
                        out=fps[:, 15 * 128:15 * 128 + 96],
                        in0=ps[:, 0:96],
                        in1=cb2_bcast(3),
                        op=ADD,
                    )
                    nc.vector.tensor_tensor(
                        out=fps[0:64, 63 * D:64 * D],
                        in0=ps[0:64, 96:128],
                        in1=cb2bs[0:64, :],
                        op=ADD,
                    )

            def gav_mms(ga_t, sub, kt):
                kw = 64 if kt == NKT - 1 else 128
                for mc in range(4):
                    nc.tensor.matmul(
                        psg[:, mc * 512:(mc + 1) * 512],
                        fps[0:kw, kt * D:(kt + 1) * D],
                        ga_t[0:kw, sub, mc * 512:(mc + 1) * 512],
                        start=(kt == 0),
                        stop=(kt == NKT - 1),
                    )

            def emit_gav_pair(dm):
                # alternate the two HWDGE rings (SP / ACT) so transfers
                # overlap across DMA boundaries
                if KCFG["ring22"]:
                    dmae = nc.sync if (dm // 2) % 2 == 0 else nc.scalar
                else:
                    dmae = nc.sync if dm % 2 == 0 else nc.scalar
                ga_t = gap.tile([128, 2, M2], BF, tag="ga")
                if dm < 31:
                    dmae.dma_start(
                        out=ga_t,
                        in_=gaT[dm * 256:(dm + 1) * 256, :].rearrange(
                            "(two p) m -> p two m", two=2
                        ),
                    )
                elif KCFG["last_split"]:
                    nc.sync.dma_start(out=ga_t[:, 0, :], in_=gaT[7936:8064, :])
                    nc.scalar.dma_start(out=ga_t[0:64, 1, :], in_=gaT[8064:8128, :])
                else:
                    dmae.dma_start(out=ga_t[:, 0, :], in_=gaT[7936:8064, :])
                    dmae.dma_start(out=ga_t[0:64, 1, :], in_=gaT[8064:8128, :])
                for half in range(2):
                    gav_mms(ga_t, half, dm * 2 + half)

            def emit_gav_quad(g):
                dmae = nc.sync if g % 2 == 0 else nc.scalar
                ga_t = gap.tile([128, 4, M2], BF, tag="ga")
                if g < 15:
                    dmae.dma_start(
                        out=ga_t,
                        in_=gaT[g * 512:(g + 1) * 512, :].rearrange(
                            "(four p) m -> p four m", four=4
                        ),
                    )
                else:
                    dmae.dma_start(
                        out=ga_t[:, 0:3, :],
                        in_=gaT[7680:8064, :].rearrange(
                            "(three p) m -> p three m", three=3
                        ),
                    )
                    (nc.sync if KCFG["last_split"] else dmae).dma_start(
                        out=ga_t[0:64, 3, :], in_=gaT[8064:8128, :]
                    )
                for sub in range(4):
                    gav_mms(ga_t, sub, g * 4 + sub)

            def emit_group(g):
                if KCFG["fp_batch"]:
                    emit_fp_tile(g)
                else:
                    for q in range(4 * g, 4 * g + 4):
                        emit_fp(q)
                if KCFG["quad"]:
                    emit_gav_quad(g)
                else:
                    emit_gav_pair(2 * g)
                    emit_gav_pair(2 * g + 1)

            def emit_weights():
                nc.gpsimd.dma_start(out=grts, in_=grT[:])
                nc.gpsimd.dma_start(
                    out=ew2s, in_=ew2t[:].rearrange("p (t e) -> p t e", t=4)
                )
                nc.gpsimd.dma_start(
                    out=ew1s, in_=ew1t[:].rearrange("p (t h) -> p t h", t=20)
                )

            nchunks = P // c1w + 1          # 16 (coarse) or 32 (fine)
            lead = KCFG["lead"] if not KCFG["fine"] else 4
            gpc = 2 if not KCFG["fine"] else 1   # gav DMA pairs per chunk step
            w_at = KCFG["w_at"] if not KCFG["fine"] else 16
            grv_at = 10 if not KCFG["fine"] else 20
            zte_at = KCFG["zte_at"] if not KCFG["fine"] else 28

            ps1 = None
            psE1_cm = None
            for pc in range(nchunks):
                emit_pair_blocks_until((pc + 1) * c1w)
                emit_c1(pc)
                if pc == w_at and KCFG["w_gpsimd"]:
                    emit_weights()
                if pc >= lead:
                    g = pc - lead
                    for q in range(4 * g * gpc // 2, 4 * (g + 1) * gpc // 2):
                        emit_fp(q)
                    for dm in range(g * gpc, (g + 1) * gpc):
                        emit_gav_pair(dm)
                if pc == grv_at and KCFG["late_init"]:
                    nc.vector.memset(ones1, 1.0)
                    make_identity(nc, ident)
                if pc == grv_at:
                    # grv in two psum rounds; r -> (kt=r//2, half=r%2);
                    # grT is r-major so the moving operand is contiguous
                    for rnd in range(2):
                        ps_grv = psGrv.tile([128, 8, N], F32, tag="grv")
                        for rr in range(16):
                            r = rnd * 16 + rr
                            base = (r % 2) * 64
                            nc.tensor.matmul(
                                ps_grv[base:base + 64, rr // 2, :],
                                afvs[:, :],
                                grts[:, r * N:(r + 1) * N],
                                start=True,
                                stop=True,
                                tile_position=(0, base),
                            )
                        nc.vector.tensor_copy(
                            zt[:, rnd * 8 * N:(rnd + 1) * 8 * N], ps_grv[:, :, :]
                        )
                    psGrv_cm.__exit__(None, None, None)
                    psE1_cm = tc.tile_pool(name="psE1", bufs=1, space="PSUM")
                    psE1 = psE1_cm.__enter__()
                    ps1 = psE1.tile([128, H], F32)
                if pc == zte_at:
                    # A1[n, h] accumulation: rank-1 eb1 + 16 grv k-tiles now,
                    # 4 gav k-tiles at the very end
                    nc.tensor.matmul(ps1, ones1, eb1rs, start=True, stop=False)
                    for kt in range(16):
                        nc.tensor.matmul(
                            ps1,
                            zt[:, kt * N:(kt + 1) * N],
                            ew1s[:, kt, :],
                            start=False,
                            stop=False,
                        )
            assert not KCFG["fine"]
            for g in range(nchunks - lead, nchunks):
                emit_group(g)

            # pack the 16 [32, n] gav pieces into 4 full 128-partition
            # k-tiles (piece r'=4t+q -> partitions q*32.., k-tile t): one
            # strided DVE op per q covers all four t at once
            if KCFG["pieces4"]:
                psg4 = psg[:].rearrange("d (t q n) -> d q t n", q=4, n=N)
                for q in range(4):
                    if KCFG["pieces22"] and q % 2 == 1:
                        nc.scalar.activation(
                            ztg[q * 32:(q + 1) * 32, :, :], psg4[:, q, :, :], IDENT
                        )
                    else:
                        nc.vector.tensor_copy(
                            ztg[q * 32:(q + 1) * 32, :, :], psg4[:, q, :, :]
                        )
            else:
                for rp in range(Ra):
                    t, q = rp // 4, rp % 4
                    dst = ztg[q * 32:(q + 1) * 32, t, :]
                    src = psg[:, rp * N:(rp + 1) * N]
                    if rp % 3 == 2:
                        nc.scalar.activation(dst, src, IDENT)
                    else:
                        nc.vector.tensor_copy(dst, src)
            for t in range(4):
                nc.tensor.matmul(
                    ps1,
                    ztg[:, t, :],
                    ew1s[:, 16 + t, :],
                    start=False,
                    stop=(t == 3),
                )
            if KCFG["gelu_split"]:
                for ht in range(4):
                    nc.scalar.activation(
                        a1[:, ht * 128:(ht + 1) * 128],
                        ps1[:, ht * 128:(ht + 1) * 128], GELU, bias=0.0, scale=1.0,
                    )
            else:
                nc.scalar.activation(a1, ps1, GELU, bias=0.0, scale=1.0)

            psE1_cm.__exit__(None, None, None)
            psA_cm.__exit__(None, None, None)
            psGav_cm.__exit__(None, None, None)

            # ---- stage 5 tail: transpose A1, final projection ----
            with tc.tile_pool(name="psE2", bufs=2, space="PSUM") as psE2:
                for ht in range(4):
                    tr = psE2.tile([128, N], BF, tag="tr")
                    nc.tensor.transpose(tr, a1[:, ht * 128:(ht + 1) * 128], ident)
                    nc.vector.tensor_copy(a1t[:, ht, :], tr)
                for ec in range(2):
                    ps2 = psE2.tile([128, N], F32, tag="aef")
                    for ht in range(4):
                        nc.tensor.matmul(
                            ps2,
                            ew2s[:, ht, ec * 128:(ec + 1) * 128],
                            a1t[:, ht, :],
                            start=(ht == 0),
                            stop=(ht == 3),
                        )
                    nc.scalar.activation(
                        aeft[:, ec, :], ps2, IDENT, bias=eb2s[:, ec:ec + 1], scale=1.0,
                    )
                    if KCFG["out_split"]:
                        nc.sync.dma_start(
                            out=out[ec * 128:(ec + 1) * 128, :], in_=aeft[:, ec, :]
                        )
                if not KCFG["out_split"]:
                    nc.sync.dma_start(
                        out=out[:].rearrange("(c e) n -> e c n", c=2), in_=aeft
                    )

    nc.compile()
    return nc


def _get_nc():
    if "nc" not in _CACHE:
        _CACHE["nc"] = _build_nc()
    return _CACHE["nc"]


def _prep_in_maps(gr, ga, afv, cw1, cb1, cw2, cb2, ew1, eb1, ew2, eb2):
    gr = np.asarray(gr, np.float32)
    ga = np.asarray(ga, np.float32)
    afv = np.asarray(afv, np.float32)
    cw1 = np.asarray(cw1, np.float32)
    cb1 = np.asarray(cb1, np.float32)
    cw2 = np.asarray(cw2, np.float32)
    cb2 = np.asarray(cb2, np.float32)
    ew1 = np.asarray(ew1, np.float32)
    eb1 = np.asarray(eb1, np.float32)
    ew2 = np.asarray(ew2, np.float32)
    eb2 = np.asarray(eb2, np.float32)

    # ew1/ew2 pretiled to [128, kt*cols] so the DMA is one contiguous
    # run per partition (row c = kt*128 + p -> partition p, block kt)
    ew1t = np.ascontiguousarray(
        ew1.reshape(20, 128, H).transpose(1, 0, 2).reshape(128, 20 * H)
        .astype(BF16NP)
    )
    ew2t = np.ascontiguousarray(
        ew2.reshape(4, 128, E).transpose(1, 0, 2).reshape(128, 4 * E)
        .astype(BF16NP)
    )
    shared = {
        "eb1r": np.ascontiguousarray(eb1.reshape(1, H).astype(BF16NP)),
        "ew1t": ew1t,
        "ew2t": ew2t,
    }
    packB = np.concatenate(
        [cw1, cw2, np.zeros((2 * A, A), np.float32)], axis=1
    ).astype(BF16NP)
    in_maps = []
    ones64 = np.ones((A, N), np.float32)
    zeros64 = np.zeros((A, N), np.float32)
    for b in range(B):
        afvT = np.ascontiguousarray(afv[b].T)  # [64, 128]
        m = dict(shared)
        # gaT: [P, (r', n)] r'-major columns
        m["gaT"] = np.ascontiguousarray(
            ga[b].transpose(1, 0, 2).reshape(M2, P).T.astype(BF16NP)
        )
        # grT: [m, (r, n)] r-major so per-r rhs slices are contiguous
        m["grT"] = np.ascontiguousarray(
            gr[b].transpose(2, 1, 0).reshape(N, G2).astype(BF16NP)
        )
        pb = packB.copy()
        pb[:, 160:224] = afv[b].astype(BF16NP)
        m["packB"] = pb
        pf = np.empty((128, 419), np.float32)
        pf[:, 0:1] = cb1.reshape(2 * A, 1)
        pf[:, 1:33] = np.broadcast_to(cb2, (2 * A, D))
        pf[:, 33:161] = np.concatenate([afvT, afvT], axis=0)
        pf[:, 161:289] = np.concatenate([ones64, afvT], axis=0)
        pf[:, 289:417] = np.concatenate([afvT, zeros64], axis=0)
        pf[:, 417:419] = eb2.reshape(2, 128).T
        m["packF"] = pf
        in_maps.append(m)
    return in_maps


def run(inputs: dict, trace: bool = False):
    """Returns ((aef, afv), exec_time_ns_or_None)."""
    nc = _get_nc()
    in_maps = _prep_in_maps(**inputs)
    res = run_bass_kernel_spmd(nc, in_maps, core_ids=list(range(B)), trace=trace)
    aef = np.stack(
        [np.ascontiguousarray(res.results[b]["out"].T) for b in range(B)], axis=0
    )
    afv = np.asarray(inputs["afv"], np.float32)
    return (aef, afv), res.exec_time_ns


def kernel(**inputs) -> np.ndarray:
    (aef, afv), _ = run(inputs, trace=False)
    return aef, afv


# revision 37
# speedup vs baseline: 1.1179x; 1.1179x over previous
"""AIMNet-style embedding kernel for 8 Trainium2 NeuronCores.

Data-parallel over the molecule batch B=8 (one molecule per core).
Host-side prep does layout transforms only (transpose ga/gr so the
contraction axis is on partitions, bf16 casts, small broadcast tables);
all FLOPs run on device.

Per-core device pipeline (molecule b):
  1. pair build:  X^T[128, 8128]  (one tensor_scalar per anchor atom i,
     split across DVE and GpSimd)
  2. combine MLP: C1^T = cw1^T @ X^T -> gelu -> G1^T ;  FP = G1^T chunks @ cw2
  3. grv:  afv^T @ grT slices  (per radial shift r)     -> Z^T k-tiles 0..15
  4. gav:  FP_k^T @ gaT k-tiles (64 accumulating steps into psum[32, 2048])
  5. embed MLP: accumulate psum[n, 512] over Z^T k-tiles (Z as stationary,
     ew1 as moving; eb1 folded in as a rank-1 matmul), gelu, PE-transpose
     A1 -> A1^T, then ew2^T @ A1^T -> AEF^T (+eb2) -> out

Stages 1/2/4 are emitted interleaved so the PE's in-order stream starts
consuming gaT tiles as soon as the first FP tiles exist, and the zt-part
of the embed accumulation runs mid-stream; only the 4 gav k-tiles of the
embed remain after the last gaT tile.
"""

import numpy as np
import ml_dtypes

import concourse.bass as bass
import concourse.mybir as mybir
import concourse.tile as tile
from concourse import bacc
from concourse.bass_utils import run_bass_kernel_spmd
from concourse.masks import make_identity

BF16NP = ml_dtypes.bfloat16
F32 = mybir.dt.float32
BF = mybir.dt.bfloat16

B, N, A = 8, 128, 64
Rr, Ra = 32, 16
P = N * (N - 1) // 2          # 8128
D = 32                        # d_pair
H, E = 512, 256
M2 = N * Ra                   # 2048 = gav output dim (r'-major: m = r'*128 + n)
G2 = N * Rr                   # 4096 = grT cols (r-major: r*128 + n)
NKT = (P + N - 1) // N        # 64 pair k-tiles (63 full + one of 64)

GELU = mybir.ActivationFunctionType.Gelu_apprx_tanh
IDENT = mybir.ActivationFunctionType.Identity
MULT = mybir.AluOpType.mult
ADD = mybir.AluOpType.add

_CACHE: dict = {}

# tunables (A/B testing); kernel defaults are the shipped config
KCFG = {
    "ga_bufs": 14,
    "pieces4": True,     # 4 strided piece-pack ops vs 16 small ones
    "gelu_split": True,  # per-chunk a1 gelu
    "out_split": True,   # per-chunk output DMA
    "w_gpsimd": True,    # bulk weights via SWDGE mid-stream (rings stay clean)
    "split32": True,     # 3:2 DVE:GpSimd pair-block split (else 2:1)
    "last_split": True,  # final DMA pair split across both rings
    "fine": False,       # 256-pair C1 chunks (faster pipeline head)
    "w_at": 3,           # loop position to issue SWDGE weight loads
    "late_init": True,   # emit ones1/ident init away from the queue head
    "lead": 2,           # chunks of slack between c1 and its gav group
    "const_scalar": False,  # consts on the scalar ring (sync ring starts with ga)
    "quad": False,       # 2MB quad gaT DMAs (4 k-tiles) instead of 1MB pairs
    "split772": False,   # 7:7:2 DVE:GpSimd:ACT pair-block split
    "pieces22": True,    # piece-pack ops split 2 DVE + 2 ACT
    "fp_batch": False,   # one psum tile + one DVE add per 4 FP chunks
    "ring22": False,     # 2-2 ring interleave for gaT pairs (else 1-1)
    "zte_at": 14,        # loop position of the zt-part embed accumulation
    "zte_spread": True,  # spread the zt-embed matmuls over pc=11..14
}


def _build_nc():
    nc = bacc.Bacc("TRN2", target_bir_lowering=False)

    gaT = nc.dram_tensor("gaT", [P, M2], BF, kind="ExternalInput")
    grT = nc.dram_tensor("grT", [N, G2], BF, kind="ExternalInput")
    # packF cols: cb1 0:1 | cb2b 1:33 | afv2 33:161 | s1 161:289 | s2 289:417 | eb2t 417:419
    packF = nc.dram_tensor("packF", [128, 419], F32, kind="ExternalInput")
    # packB cols: cw1 0:128 | cw2 128:160 | afv 160:224
    packB = nc.dram_tensor("packB", [128, 224], BF, kind="ExternalInput")
    eb1r = nc.dram_tensor("eb1r", [1, H], BF, kind="ExternalInput")
    ew1t = nc.dram_tensor("ew1t", [128, 20 * H], BF, kind="ExternalInput")
    ew2t = nc.dram_tensor("ew2t", [128, 4 * E], BF, kind="ExternalInput")
    out = nc.dram_tensor("out", [E, N], F32, kind="ExternalOutput")

    with tile.TileContext(nc) as tc:
        with (
            tc.tile_pool(name="const", bufs=1) as cp,
            tc.tile_pool(name="big", bufs=1) as bp,
            tc.tile_pool(name="ga", bufs=KCFG["ga_bufs"]) as gap,
        ):
            # ---- constants (host-packed: 3 small DMAs) ----
            cdma = nc.scalar if KCFG["const_scalar"] else nc.sync
            packFs = cp.tile([128, 419], F32)
            cdma.dma_start(out=packFs, in_=packF[:])
            packBs = cp.tile([128, 224], BF)
            cdma.dma_start(out=packBs, in_=packB[:])
            eb1rs = cp.tile([1, H], BF)
            cdma.dma_start(out=eb1rs, in_=eb1r[:])
            cb1s = packFs[:, 0:1]
            cb2bs = packFs[:, 1:33]
            afv2s = packFs[:, 33:161]
            s1s = packFs[:, 161:289]
            s2s = packFs[:, 289:417]
            eb2s = packFs[:, 417:419]
            cw1s = packBs[:, 0:128]
            cw2s = packBs[:, 128:160]
            afvs = packBs[:, 160:224]
            # bulk weights (host-pretiled: contiguous per-partition runs)
            grts = cp.tile([128, G2], BF)
            ew2s = cp.tile([128, 4, E], BF)
            ew1s = cp.tile([128, 20, H], BF)
            wdma = nc.gpsimd if KCFG["w_gpsimd"] else nc.scalar
            if not KCFG["w_gpsimd"]:
                wdma.dma_start(out=grts, in_=grT[:])
                wdma.dma_start(
                    out=ew2s, in_=ew2t[:].rearrange("p (t e) -> p t e", t=4)
                )
                wdma.dma_start(
                    out=ew1s, in_=ew1t[:].rearrange("p (t h) -> p t h", t=20)
                )

            ones1 = cp.tile([1, N], BF)
            ident = cp.tile([128, 128], BF)
            if not KCFG["late_init"]:
                nc.vector.memset(ones1, 1.0)
                make_identity(nc, ident)

            # ---- persistent intermediates ----
            xt = bp.tile([128, P], BF)           # X^T  (pair features)
            g1t = bp.tile([128, P], BF)          # gelu(C1)^T
            fps = bp.tile([128, NKT * D], BF)    # FP, k-tile q at cols [q*32, q*32+32)
            zt = bp.tile([128, 16 * N], BF)      # Z^T grv part, k-tile kt at cols kt*128
            ztg = bp.tile([128, 4, N], BF)       # Z^T gav part, packed 4 pieces/k-tile
            a1 = bp.tile([128, H], BF)           # A1 [n, h]
            a1t = bp.tile([128, 4, N], BF)       # A1^T, h-chunk ht at [:, ht, :]
            aeft = bp.tile([128, 2, N], F32)     # AEF^T chunks

            # ---- stages 1+2+4 interleaved ----
            # Pair block i (i=0..126) covers pairs (i, j) j=i+1..127 (width
            # 127-i).  Emission is pipelined per 512-pair C1 chunk, and the
            # gav matmuls for group g (FP chunks 4g..4g+3 = DMA pairs 2g,
            # 2g+1) are emitted right after the FP tiles they need, so the
            # PE's in-order stream never parks gav work behind the whole
            # pair-build chain.
            offs = np.concatenate([[0], np.cumsum(N - 1 - np.arange(N - 1))])
            next_blk = 0

            def emit_pair_blocks_until(cov):
                nonlocal next_blk
                while next_blk < N - 1 and offs[next_blk] < cov:
                    i = next_blk
                    if KCFG["split772"] and i % 16 >= 14:
                        nc.scalar.activation(
                            xt[:, offs[i]:offs[i + 1]],
                            afv2s[:, i + 1:N],
                            IDENT,
                            bias=s2s[:, i:i + 1],
                            scale=s1s[:, i:i + 1],
                        )
                        next_blk += 1
                        continue
                    if KCFG["split772"]:
                        eng = nc.gpsimd if i % 2 == 1 else nc.vector
                    elif KCFG["split32"]:
                        eng = nc.gpsimd if i % 5 in (2, 4) else nc.vector
                    else:
                        eng = nc.gpsimd if i % 3 == 2 else nc.vector
                    eng.tensor_scalar(
                        out=xt[:, offs[i]:offs[i + 1]],
                        in0=afv2s[:, i + 1:N],
                        scalar1=s1s[:, i:i + 1],
                        scalar2=s2s[:, i:i + 1],
                        op0=MULT,
                        op1=ADD,
                    )
                    next_blk += 1

            # PSUM pool lifetimes (explicit, LIFO):
            #   psGav (4 banks) spans the whole stream;
            #   psA (c1/fp shared tag, 2 banks) spans stages 1+2;
            #   psGrv (2 banks) closes after grv, freeing room for ps1 (1).
            psGav_cm = tc.tile_pool(name="psGav", bufs=1, space="PSUM")
            psGav = psGav_cm.__enter__()
            psA_cm = tc.tile_pool(name="psA", bufs=2, space="PSUM")
            psA = psA_cm.__enter__()
            psGrv_cm = tc.tile_pool(name="psGrv", bufs=1, space="PSUM")
            psGrv = psGrv_cm.__enter__()

            psg = psGav.tile([32, M2], F32)

            c1w = 256 if KCFG["fine"] else 512

            def emit_c1(pc):
                w = min(c1w, P - pc * c1w)
                ps = psA.tile([128, 512], F32, tag="c1")
                nc.tensor.matmul(
                    ps[:, 0:w], cw1s[:, :], xt[:, pc * c1w:pc * c1w + w],
                    start=True, stop=True,
                )
                nc.scalar.activation(
                    g1t[:, pc * c1w:pc * c1w + w], ps[:, 0:w], GELU,
                    bias=cb1s[:, 0:1], scale=1.0,
                )

            def emit_fp(q):
                kw = min(128, P - q * 128)
                ps = psA.tile([128, 512], F32, tag="c1")
                nc.tensor.matmul(
                    ps[0:kw, 0:D], g1t[:, q * 128:q * 128 + kw], cw2s[:, :],
                    start=True, stop=True,
                )
                nc.vector.tensor_tensor(
                    out=fps[0:kw, q * D:(q + 1) * D],
                    in0=ps[0:kw, 0:D],
                    in1=cb2bs[0:kw, :],
                    op=ADD,
                )

            def cb2_bcast(reps, rows=128):
                ap = cb2bs[0:rows, :]
                import copy as _copy
                b = bass.AP(tensor=ap.tensor, offset=ap.offset,
                            ap=[ap.ap[0], [0, reps], ap.ap[1]])
                return b

            def emit_fp_tile(g):
                # 4 FP chunks (4g..4g+3) into one psum tile, one batched add
                ps = psA.tile([128, 512], F32, tag="c1")
                for sub in range(4):
                    q = 4 * g + sub
                    kw = min(128, P - q * 128)
                    nc.tensor.matmul(
                        ps[0:kw, sub * D:(sub + 1) * D],
                        g1t[:, q * 128:q * 128 + kw], cw2s[:, :],
                        start=True, stop=True,
                    )
                if g < 15:
                    nc.vector.tensor_tensor(
                        out=fps[:, g * 128:(g + 1) * 128],
                        in0=ps[:, 0:128],
                        in1=cb2_bcast(4),
                        op=ADD,
                    )
                else:
                    nc.vector.tensor_tensor(
                        out=fps[:, 15 * 128:15 * 128 + 96],
                        in0=ps[:, 0:96],
                        in1=cb2_bcast(3),
                        op=ADD,
                    )
                    nc.vector.tensor_tensor(
                        out=fps[0:64, 63 * D:64 * D],
                        in0=ps[0:64, 96:128],
                        in1=cb2bs[0:64, :],
                        op=ADD,
                    )

            def gav_mms(ga_t, sub, kt):
                kw = 64 if kt == NKT - 1 else 128
                for mc in range(4):
                    nc.tensor.matmul(
                        psg[:, mc * 512:(mc + 1) * 512],
                        fps[0:kw, kt * D:(kt + 1) * D],
                        ga_t[0:kw, sub, mc * 512:(mc + 1) * 512],
                        start=(kt == 0),
                        stop=(kt == NKT - 1),
                    )

            def emit_gav_pair(dm):
                # alternate the two HWDGE rings (SP / ACT) so transfers
                # overlap across DMA boundaries
                if KCFG["ring22"]:
                    dmae = nc.sync if (dm // 2) % 2 == 0 else nc.scalar
                else:
                    dmae = nc.sync if dm % 2 == 0 else nc.scalar
                ga_t = gap.tile([128, 2, M2], BF, tag="ga")
                if dm < 31:
                    dmae.dma_start(
                        out=ga_t,
                        in_=gaT[dm * 256:(dm + 1) * 256, :].rearrange(
                            "(two p) m -> p two m", two=2
                        ),
                    )
                elif KCFG["last_split"]:
                    nc.sync.dma_start(out=ga_t[:, 0, :], in_=gaT[7936:8064, :])
                    nc.scalar.dma_start(out=ga_t[0:64, 1, :], in_=gaT[8064:8128, :])
                else:
                    dmae.dma_start(out=ga_t[:, 0, :], in_=gaT[7936:8064, :])
                    dmae.dma_start(out=ga_t[0:64, 1, :], in_=gaT[8064:8128, :])
                for half in range(2):
                    gav_mms(ga_t, half, dm * 2 + half)

            def emit_gav_quad(g):
                dmae = nc.sync if g % 2 == 0 else nc.scalar
                ga_t = gap.tile([128, 4, M2], BF, tag="ga")
                if g < 15:
                    dmae.dma_start(
                        out=ga_t,
                        in_=gaT[g * 512:(g + 1) * 512, :].rearrange(
                            "(four p) m -> p four m", four=4
                        ),
                    )
                else:
                    dmae.dma_start(
                        out=ga_t[:, 0:3, :],
                        in_=gaT[7680:8064, :].rearrange(
                            "(three p) m -> p three m", three=3
                        ),
                    )
                    (nc.sync if KCFG["last_split"] else dmae).dma_start(
                        out=ga_t[0:64, 3, :], in_=gaT[8064:8128, :]
                    )
                for sub in range(4):
                    gav_mms(ga_t, sub, g * 4 + sub)

            def emit_group(g):
                if KCFG["fp_batch"]:
                    emit_fp_tile(g)
                else:
                    for q in range(4 * g, 4 * g + 4):
                        emit_fp(q)
                if KCFG["quad"]:
                    emit_gav_quad(g)
                else:
                    emit_gav_pair(2 * g)
                    emit_gav_pair(2 * g + 1)

            def emit_weights():
                nc.gpsimd.dma_start(out=grts, in_=grT[:])
                nc.gpsimd.dma_start(
                    out=ew2s, in_=ew2t[:].rearrange("p (t e) -> p t e", t=4)
                )
                nc.gpsimd.dma_start(
                    out=ew1s, in_=ew1t[:].rearrange("p (t h) -> p t h", t=20)
                )

            nchunks = P // c1w + 1          # 16 (coarse) or 32 (fine)
            lead = KCFG["lead"] if not KCFG["fine"] else 4
            gpc = 2 if not KCFG["fine"] else 1   # gav DMA pairs per chunk step
            w_at = KCFG["w_at"] if not KCFG["fine"] else 16
            grv_at = 10 if not KCFG["fine"] else 20
            zte_at = KCFG["zte_at"] if not KCFG["fine"] else 28

            ps1 = None
            psE1_cm = None
            for pc in range(nchunks):
                emit_pair_blocks_until((pc + 1) * c1w)
                emit_c1(pc)
                if pc == w_at and KCFG["w_gpsimd"]:
                    emit_weights()
                if pc >= lead:
                    g = pc - lead
                    for q in range(4 * g * gpc // 2, 4 * (g + 1) * gpc // 2):
                        emit_fp(q)
                    for dm in range(g * gpc, (g + 1) * gpc):
                        emit_gav_pair(dm)
                if pc == grv_at and KCFG["late_init"]:
                    nc.vector.memset(ones1, 1.0)
                    make_identity(nc, ident)
                if pc == grv_at:
                    # grv in two psum rounds; r -> (kt=r//2, half=r%2);
                    # grT is r-major so the moving operand is contiguous
                    for rnd in range(2):
                        ps_grv = psGrv.tile([128, 8, N], F32, tag="grv")
                        for rr in range(16):
                            r = rnd * 16 + rr
                            base = (r % 2) * 64
                            nc.tensor.matmul(
                                ps_grv[base:base + 64, rr // 2, :],
                                afvs[:, :],
                                grts[:, r * N:(r + 1) * N],
                                start=True,
                                stop=True,
                                tile_position=(0, base),
                            )
                        nc.vector.tensor_copy(
                            zt[:, rnd * 8 * N:(rnd + 1) * 8 * N], ps_grv[:, :, :]
                        )
                    psGrv_cm.__exit__(None, None, None)
                    psE1_cm = tc.tile_pool(name="psE1", bufs=1, space="PSUM")
                    psE1 = psE1_cm.__enter__()
                    ps1 = psE1.tile([128, H], F32)
                if KCFG["zte_spread"] and pc in (11, 12, 13, 14):
                    # spread the zt-embed accumulation so each insertion is
                    # absorbed by the PE's per-pair slack vs the DMA pace
                    kts = {11: range(0, 4), 12: range(4, 8),
                           13: range(8, 12), 14: range(12, 16)}[pc]
                    if pc == 11:
                        nc.tensor.matmul(ps1, ones1, eb1rs, start=True, stop=False)
                    for kt in kts:
                        nc.tensor.matmul(
                            ps1,
                            zt[:, kt * N:(kt + 1) * N],
                            ew1s[:, kt, :],
                            start=False,
                            stop=False,
                        )
                elif not KCFG["zte_spread"] and pc == zte_at:
                    # A1[n, h] accumulation: rank-1 eb1 + 16 grv k-tiles now,
                    # 4 gav k-tiles at the very end
                    nc.tensor.matmul(ps1, ones1, eb1rs, start=True, stop=False)
                    for kt in range(16):
                        nc.tensor.matmul(
                            ps1,
                            zt[:, kt * N:(kt + 1) * N],
                            ew1s[:, kt, :],
                            start=False,
                            stop=False,
                        )
            assert not KCFG["fine"]
            for g in range(nchunks - lead, nchunks):
                emit_group(g)

            # pack the 16 [32, n] gav pieces into 4 full 128-partition
            # k-tiles (piece r'=4t+q -> partitions q*32.., k-tile t): one
            # strided DVE op per q covers all four t at once
            if KCFG["pieces4"]:
                psg4 = psg[:].rearrange("d (t q n) -> d q t n", q=4, n=N)
                for q in range(4):
                    if KCFG["pieces22"] and q % 2 == 1:
                        nc.scalar.activation(
                            ztg[q * 32:(q + 1) * 32, :, :], psg4[:, q, :, :], IDENT
                        )
                    else:
                        nc.vector.tensor_copy(
                            ztg[q * 32:(q + 1) * 32, :, :], psg4[:, q, :, :]
                        )
            else:
                for rp in range(Ra):
                    t, q = rp // 4, rp % 4
                    dst = ztg[q * 32:(q + 1) * 32, t, :]
                    src = psg[:, rp * N:(rp + 1) * N]
                    if rp % 3 == 2:
                        nc.scalar.activation(dst, src, IDENT)
                    else:
                        nc.vector.tensor_copy(dst, src)
            for t in range(4):
                nc.tensor.matmul(
                    ps1,
                    ztg[:, t, :],
                    ew1s[:, 16 + t, :],
                    start=False,
                    stop=(t == 3),
                )
            if KCFG["gelu_split"]:
                for ht in range(4):
                    nc.scalar.activation(
                        a1[:, ht * 128:(ht + 1) * 128],
                        ps1[:, ht * 128:(ht + 1) * 128], GELU, bias=0.0, scale=1.0,
                    )
            else:
                nc.scalar.activation(a1, ps1, GELU, bias=0.0, scale=1.0)

            psE1_cm.__exit__(None, None, None)
            psA_cm.__exit__(None, None, None)
            psGav_cm.__exit__(None, None, None)

            # ---- stage 5 tail: transpose A1, final projection ----
            with tc.tile_pool(name="psE2", bufs=2, space="PSUM") as psE2:
                for ht in range(4):
                    tr = psE2.tile([128, N], BF, tag="tr")
                    nc.tensor.transpose(tr, a1[:, ht * 128:(ht + 1) * 128], ident)
                    nc.vector.tensor_copy(a1t[:, ht, :], tr)
                for ec in range(2):
                    ps2 = psE2.tile([128, N], F32, tag="aef")
                    for ht in range(4):
                        nc.tensor.matmul(
                            ps2,
                            ew2s[:, ht, ec * 128:(ec + 1) * 128],
                            a1t[:, ht, :],
                            start=(ht == 0),
                            stop=(ht == 3),
                        )
                    nc.scalar.activation(
                        aeft[:, ec, :], ps2, IDENT, bias=eb2s[:, ec:ec + 1], scale=1.0,
                    )
                    if KCFG["out_split"]:
                        nc.sync.dma_start(
                            out=out[ec * 128:(ec + 1) * 128, :], in_=aeft[:, ec, :]
                        )
                if not KCFG["out_split"]:
                    nc.sync.dma_start(
                        out=out[:].rearrange("(c e) n -> e c n", c=2), in_=aeft
                    )

    nc.compile()
    return nc


def _get_nc():
    if "nc" not in _CACHE:
        _CACHE["nc"] = _build_nc()
    return _CACHE["nc"]


def _prep_in_maps(gr, ga, afv, cw1, cb1, cw2, cb2, ew1, eb1, ew2, eb2):
    gr = np.asarray(gr, np.float32)
    ga = np.asarray(ga, np.float32)
    afv = np.asarray(afv, np.float32)
    cw1 = np.asarray(cw1, np.float32)
    cb1 = np.asarray(cb1, np.float32)
    cw2 = np.asarray(cw2, np.float32)
    cb2 = np.asarray(cb2, np.float32)
    ew1 = np.asarray(ew1, np.float32)
    eb1 = np.asarray(eb1, np.float32)
    ew2 = np.asarray(ew2, np.float32)
    eb2 = np.asarray(eb2, np.float32)

    # ew1/ew2 pretiled to [128, kt*cols] so the DMA is one contiguous
    # run per partition (row c = kt*128 + p -> partition p, block kt)
    ew1t = np.ascontiguousarray(
        ew1.reshape(20, 128, H).transpose(1, 0, 2).reshape(128, 20 * H)
        .astype(BF16NP)
    )
    ew2t = np.ascontiguousarray(
        ew2.reshape(4, 128, E).transpose(1, 0, 2).reshape(128, 4 * E)
        .astype(BF16NP)
    )
    shared = {
        "eb1r": np.ascontiguousarray(eb1.reshape(1, H).astype(BF16NP)),
        "ew1t": ew1t,
        "ew2t": ew2t,
    }
    packB = np.concatenate(
        [cw1, cw2, np.zeros((2 * A, A), np.float32)], axis=1
    ).astype(BF16NP)
    in_maps = []
    ones64 = np.ones((A, N), np.float32)
    zeros64 = np.zeros((A, N), np.float32)
    for b in range(B):
        afvT = np.ascontiguousarray(afv[b].T)  # [64, 128]
        m = dict(shared)
        # gaT: [P, (r', n)] r'-major columns
        m["gaT"] = np.ascontiguousarray(
            ga[b].transpose(1, 0, 2).reshape(M2, P).T.astype(BF16NP)
        )
        # grT: [m, (r, n)] r-major so per-r rhs slices are contiguous
        m["grT"] = np.ascontiguousarray(
            gr[b].transpose(2, 1, 0).reshape(N, G2).astype(BF16NP)
        )
        pb = packB.copy()
        pb[:, 160:224] = afv[b].astype(BF16NP)
        m["packB"] = pb
        pf = np.empty((128, 419), np.float32)
        pf[:, 0:1] = cb1.reshape(2 * A, 1)
        pf[:, 1:33] = np.broadcast_to(cb2, (2 * A, D))
        pf[:, 33:161] = np.concatenate([afvT, afvT], axis=0)
        pf[:, 161:289] = np.concatenate([ones64, afvT], axis=0)
        pf[:, 289:417] = np.concatenate([afvT, zeros64], axis=0)
        pf[:, 417:419] = eb2.reshape(2, 128).T
        m["packF"] = pf
        in_maps.append(m)
    return in_maps


def run(inputs: dict, trace: bool = False):
    """Returns ((aef, afv), exec_time_ns_or_None)."""
    nc = _get_nc()
    in_maps = _prep_in_maps(**inputs)
    res = run_bass_kernel_spmd(nc, in_maps, core_ids=list(range(B)), trace=trace)
    aef = np.stack(
        [np.ascontiguousarray(res.results[b]["out"].T) for b in range(B)], axis=0
    )
    afv = np.asarray(inputs["afv"], np.float32)
    return (aef, afv), res.exec_time_ns


def kernel(**inputs) -> np.ndarray:
    (aef, afv), _ = run(inputs, trace=False)
    return aef, afv
